# revision 36
# baseline (speedup 1.0000x reference)
"""Trainium2 Bass kernel for nn_Attention_27943057228498 (sparse token-pruning
attention, ViT-style EViT).

Strategy: pure data parallelism over batch — 32 batches over 8 NeuronCores,
4 per core, no collectives.

Numerics: the top-k token selection compares attention diagonal values whose
boundary gaps are as small as ~6e-6 relative, so everything feeding the
ranking (QK projection, scores, softmax row sums, diagonal) must be accurate
to ~1e-6. fp32 matmuls cost 4 cycles/row on the PE (2 half-speed passes), so
instead the exact math runs as fp16 hi/lo split products at 1 cycle/row:

  - Q/K projection: q = whi@xhi + wlo@xhi + whi@xlo  (3 products; the
    dropped wlo@xlo term is ~2^-22 relative).  HW-measured rms error of
    this scheme vs fp64: 1.1e-6 (vs 1.6e-7 for fp32 matmul) — safely
    below the 6e-6 ranking gap.
  - scores: the fp32 q from PSUM is split exactly into qhi + qlo (fp16
    pair); k likewise into a stacked tile khl = [khi; klo].  Per head:
    s = khl^T @ [qhi; qhi] + khl^T @ [qlo; qlo]
      = (khi+klo)^T (qhi+qlo)   — the exact 4-product expansion,
    two K=128 fp16 matmuls per (head, token-tile) instead of the fp32
    form (which costs 4 cycles/row and gets no tile_position
    concurrency on real HW).
  - attention diagonal (ranking signal): qkm = ps_q * k32 with ps_q read
    directly from the projection PSUM (exact fp32), reduced per head with
    a one-hot selector matmul.
  - row sums: fp32 accumulator over exp tiles + ones^T matmul (exact).

The output path (V projection, attn@V, output projection) only needs ~1e-3,
so it runs in plain fp16; the residual add and the final output are fp16
as well (5e-4 rounding on a 2e-2 budget).

Token selection without sorting: per batch, token score a_j gets rank
R_j = #{i: a_i > a_j} via compare + row-reduce; keep = R < num_kept (CLS is
forced kept by pinning a_0 = +1e30); output positions are an inclusive
prefix sum over token index computed entirely in the [128 x 5] partition
layout: a triangular-ones matmul gives per-tile partition prefixes and a
5-wide scan of the tile sums supplies the cross-tile offsets (this avoids
two high-latency DRAM round-trips that a row-layout scan would need).
Rows are emitted with an indirect-DMA scatter whose out-of-bounds indices
(dropped tokens) are silently discarded.
"""

import numpy as np

import concourse.bass as bass
import concourse.bass_isa as bass_isa
import concourse.tile as tile
import concourse.mybir as mybir
from concourse import bacc
from concourse.bass_utils import run_bass_kernel_spmd

# ── problem constants ────────────────────────────────────────────────
B, N, C = 32, 577, 768
H = 12
HD = C // H              # 64
NCORES = 8
BL = B // NCORES         # 4 batches per core
SCALE = HD ** -0.5       # 0.125 (exact power of two)

P = 128
TOK_TILES = [(0, 128), (128, 128), (256, 128), (384, 128), (512, 65)]  # 577
CT = C // P              # 6 channel tiles
NPAD = 640               # 577 padded to 5*128 for the rank machinery
BIG = 1.0e9              # scatter index for dropped rows (exact in fp32)
NEG = -1.0e30            # pad value below any real score

F32 = mybir.dt.float32
F16 = mybir.dt.float16
U32 = mybir.dt.uint32
OP = mybir.AluOpType
ACTF = mybir.ActivationFunctionType


def _dedupe_ldweights(nc):
    """Remove back-to-back duplicate PE Ldweights (same weights AP + array
    tile) so repeated matmuls on one stationary operand pay one load.

    Only deletes an Ldweights when (a) it has no semaphore waits/updates of
    its own, and (b) the PE weight state for its array-tile region is
    provably identical (no intervening Ldweights / self-loading Matmult
    overlapping that region).
    """

    def region(inst):
        tp = inst.tile_position or (0, 0)
        ts = inst.tile_size or (128, 128)
        return (tp[0], tp[0] + ts[0], tp[1], tp[1] + ts[1])

    def overlaps(r1, r2):
        return r1[0] < r2[1] and r2[0] < r1[1] and r1[2] < r2[3] and r2[2] < r1[3]

    removed = 0
    for blk in nc.m.functions[0].blocks:
        state = []   # list of (region, signature)
        keep_list = []
        for inst in blk.instructions:
            if not isinstance(inst, (mybir.InstLdweights, mybir.InstMatmult)):
                keep_list.append(inst)
                continue
            if isinstance(inst, mybir.InstLdweights):
                sig = (str(inst.ins[0]), inst.tile_position, inst.tile_size,
                       inst.is_transpose)
                r = region(inst)
                if (not inst.has_wait() and not inst.has_update()
                        and any(overlaps(r, r2) and s2 == sig and r2 == r
                                for r2, s2 in state)):
                    removed += 1
                    continue     # drop duplicate load
                state = [(r2, s2) for r2, s2 in state if not overlaps(r, r2)]
                state.append((r, sig))
                keep_list.append(inst)
            else:
                # self-loading matmul (fp32) clobbers its region
                if getattr(inst, "ldweights", None) is not False:
                    r = region(inst)
                    state = [(r2, s2) for r2, s2 in state
                             if not overlaps(r, r2)]
                keep_list.append(inst)
        if removed:
            blk.instructions[:] = keep_list
    return removed


def _chunks(total, limit=512):
    out = []
    c0 = 0
    while c0 < total:
        cn = min(limit, total - c0)
        out.append((c0, cn))
        c0 += cn
    return out


def build(num_kept: int):
    nc = bacc.Bacc("TRN2", target_bir_lowering=False, debug=False,
                   num_devices=NCORES)

    xhi_e = nc.dram_tensor("xhi", [BL, C, N], F16, kind="ExternalInput")
    xlo_e = nc.dram_tensor("xlo", [BL, C, N], F16, kind="ExternalInput")
    xr_e = nc.dram_tensor("xr", [BL, N, C], F16, kind="ExternalInput")
    whiq_e = nc.dram_tensor("whiq", [C, C], F16, kind="ExternalInput")
    wloq_e = nc.dram_tensor("wloq", [C, C], F16, kind="ExternalInput")
    whik_e = nc.dram_tensor("whik", [C, C], F16, kind="ExternalInput")
    wlok_e = nc.dram_tensor("wlok", [C, C], F16, kind="ExternalInput")
    wv16_e = nc.dram_tensor("wv16", [C, C], F16, kind="ExternalInput")
    wp16_e = nc.dram_tensor("wp16", [C, C], F16, kind="ExternalInput")
    hsel_e = nc.dram_tensor("hsel", [C, H], F32, kind="ExternalInput")
    tri_e = nc.dram_tensor("tri", [P, P], F16, kind="ExternalInput")
    out_e = nc.dram_tensor("out", [BL, num_kept, C], F16, kind="ExternalOutput")
    out_flat = out_e.ap().rearrange("b n c -> (b n) c")

    from contextlib import ExitStack
    with tile.TileContext(nc) as tc, ExitStack() as ctx:
        wpool = ctx.enter_context(tc.tile_pool(name="weights", bufs=1))
        xpool = ctx.enter_context(tc.tile_pool(name="x", bufs=1))
        qkpool = ctx.enter_context(tc.tile_pool(name="qk", bufs=1))
        qtpool = ctx.enter_context(tc.tile_pool(name="qtmp", bufs=1))
        vpool = ctx.enter_context(tc.tile_pool(name="v", bufs=1))
        opool = ctx.enter_context(tc.tile_pool(name="o", bufs=1))
        ppool = ctx.enter_context(tc.tile_pool(name="p", bufs=2))
        spool = ctx.enter_context(tc.tile_pool(name="small", bufs=1))
        ypool = ctx.enter_context(tc.tile_pool(name="y", bufs=2))
        dpool = ctx.enter_context(tc.tile_pool(name="dram", bufs=2,
                                               space="DRAM"))
        pspool = ctx.enter_context(tc.tile_pool(name="ps", bufs=3,
                                                space="PSUM"))
        psav = ctx.enter_context(tc.tile_pool(name="psav", bufs=1,
                                              space="PSUM"))

        # ── resident weights / constants ─────────────────────────────
        whiq_t, wloq_t, whik_t, wlok_t = [], [], [], []
        wv_t, wp_t, hsel_t = [], [], []
        def load_weights():
            for name, src, dst in (("whiq", whiq_e, whiq_t),
                                   ("wloq", wloq_e, wloq_t),
                                   ("whik", whik_e, whik_t),
                                   ("wlok", wlok_e, wlok_t)):
                for i in range(CT):
                    w = wpool.tile([P, C], F16, tag=f"{name}{i}",
                                   name=f"{name}{i}")
                    nc.scalar.dma_start(w[:], src.ap()[i * P:(i + 1) * P, :])
                    dst.append(w)
            for i in range(CT):
                w5 = wpool.tile([P, H], F32, tag=f"hs{i}", name=f"hs{i}")
                nc.sync.dma_start(w5[:], hsel_e.ap()[i * P:(i + 1) * P, :])
                hsel_t.append(w5)
            for i in range(CT):
                w3 = wpool.tile([P, C], F16, tag=f"wv{i}", name=f"wv{i}")
                nc.scalar.dma_start(w3[:], wv16_e.ap()[i * P:(i + 1) * P, :])
                wv_t.append(w3)
            for i in range(CT):
                w4 = wpool.tile([P, C], F16, tag=f"wp{i}", name=f"wp{i}")
                nc.scalar.dma_start(w4[:], wp16_e.ap()[i * P:(i + 1) * P, :])
                wp_t.append(w4)
        zrow = wpool.tile([1, NPAD], F32, tag="zrow")
        nc.vector.memset(zrow[:], 0.0)
        tri_t = wpool.tile([P, P], F16, tag="tri")
        nc.sync.dma_start(tri_t[:], tri_e.ap()[:, :])
        ones128 = wpool.tile([P, 1], F32, tag="ones128")
        nc.vector.memset(ones128[:], 1.0)

        def load_x(b):
            hi, lo = [], []
            for i in range(CT):
                t1 = xpool.tile([P, N], F16, tag=f"xhi{i}", name=f"xhi{i}")
                nc.sync.dma_start(t1[:], xhi_e.ap()[b, i * P:(i + 1) * P, :])
                hi.append(t1)
            for i in range(CT):
                t2 = xpool.tile([P, N], F16, tag=f"xlo{i}", name=f"xlo{i}")
                nc.sync.dma_start(t2[:], xlo_e.ap()[b, i * P:(i + 1) * P, :])
                lo.append(t2)
            return hi, lo

        # ── Q/K projection emission, one 128-channel (2-head) block ──
        # Per mo-tile: q PSUM then k PSUM; the exact diagonal signal
        # qkm = ps_q * k32 is reduced with the hsel selector matmul
        # while ps_q is alive.  q splits into a combined hi|lo tile
        # (hi = cols 0:N, lo = cols N:2N) that is DMA-replicated per
        # head into qrep = [[qhi|qlo]; [qhi|qlo]]; k splits directly
        # into stacked [khi;klo] (or [klo;khi]) tiles.
        def new_proj_state():
            sd_ps = psav.tile([P, NPAD], F32, tag="avps", name="sd_ps")
            return {"qrep": {}, "khl": {}, "sd_ps": sd_ps}

        def emit_proj_mo(stt, xpair, mo):
            xhi_t, xlo_t = xpair
            qrep, khl, sd_ps = stt["qrep"], stt["khl"], stt["sd_ps"]

            def proj_ps(w_hi, w_lo):
                ps = pspool.tile([P, C], F32, tag="bigps")
                for prod, (wt, xt) in enumerate(
                        ((w_hi, xhi_t), (w_lo, xhi_t), (w_hi, xlo_t))):
                    for kc in range(CT):
                        for (c0, cn) in _chunks(N):
                            nc.tensor.matmul(
                                ps[:, c0:c0 + cn],
                                lhsT=wt[kc][:, mo * P:(mo + 1) * P],
                                rhs=xt[kc][:, c0:c0 + cn],
                                start=(prod == 0 and kc == 0),
                                stop=(prod == 2 and kc == CT - 1))
                return ps

            if True:
                ps_q = proj_ps(whiq_t, wloq_t)
                ps_k = proj_ps(whik_t, wlok_t)

                # exact diagonal contribution from this channel block
                # (DVE cannot read two PSUM operands, so stage k in SBUF)
                k32 = qtpool.tile([P, N], F32, tag="k32", name=f"k32_{mo}")
                nc.scalar.copy(k32[:], ps_k[:, :N])
                qkm = qkpool.tile([P, NPAD], F32, tag="qkm", bufs=1)
                nc.vector.tensor_mul(qkm[:, :N], ps_q[:, :N], k32[:])
                for (c0, cn) in _chunks(N):
                    nc.tensor.matmul(
                        sd_ps[:H, c0:c0 + cn],
                        lhsT=hsel_t[mo][:],
                        rhs=qkm[:, c0:c0 + cn],
                        start=(mo == 0), stop=(mo == CT - 1))

                # q: hi/lo split into one combined tile (hi|lo columns),
                # then per-head replication with 2 DMAs per head
                qhl = qtpool.tile([P, 2 * N], F16, tag="qhl", bufs=2,
                                  name=f"qhl{mo}")
                nc.scalar.copy(qhl[:, 0:N], ps_q[:, :N])
                nc.vector.tensor_tensor(qhl[:, N:2 * N], ps_q[:, :N],
                                        qhl[:, 0:N], OP.subtract)
                for par in (0, 1):
                    h = 2 * mo + par
                    r = par * HD
                    t_r = qkpool.tile([P, 2 * N], F16, tag=f"qrep{h}",
                                      name=f"qrep{h}")
                    nc.sync.dma_start(t_r[0:HD, :], qhl[r:r + HD, :])
                    nc.sync.dma_start(t_r[HD:P, :], qhl[r:r + HD, :])
                    qrep[h] = t_r

                # k: stacked hi/lo per head.  Even head: [khi; klo] with
                # the hi half a lanewise copy; odd head: [klo; khi].
                h0, h1 = 2 * mo, 2 * mo + 1
                st0 = qkpool.tile([P, N], F16, tag=f"khl{h0}",
                                  name=f"khl{h0}")
                st1 = qkpool.tile([P, N], F16, tag=f"khl{h1}",
                                  name=f"khl{h1}")
                nc.scalar.copy(st0[0:HD, :], ps_k[0:HD, :N])
                nc.scalar.copy(st1[HD:P, :], ps_k[HD:P, :N])
                k_lo = qtpool.tile([P, N], F16, tag="k_lo", bufs=2,
                                   name=f"klo{mo}")
                nc.vector.tensor_tensor(k_lo[0:HD, :], ps_k[0:HD, :N],
                                        st0[0:HD, :], OP.subtract)
                nc.vector.tensor_tensor(k_lo[HD:P, :], ps_k[HD:P, :N],
                                        st1[HD:P, :], OP.subtract)
                nc.scalar.dma_start(st0[HD:P, :], k_lo[0:HD, :])
                nc.scalar.dma_start(st1[0:HD, :], k_lo[HD:P, :])
                khl[h0] = st0
                khl[h1] = st1

        next_x = load_x(0)
        load_weights()
        carried = None
        for b in range(BL):
            xhi_t, xlo_t = next_x
            if carried is None:
                stt = new_proj_state()
                mos = range(CT)
            else:
                stt = carried
                carried = None
                mos = range(2, CT)
            for mo in mos:
                emit_proj_mo(stt, (xhi_t, xlo_t), mo)
            qrep, khl, sd_ps = stt["qrep"], stt["khl"], stt["sd_ps"]

            sd_sb = spool.tile([H, N], F32, tag="sd_sb")
            nc.scalar.copy(sd_sb[:], sd_ps[:H, :N])

            # ── V projection (fp16) → v16[mt] = (m, C) ───────────────
            v16 = []
            for mt, (t0, tn) in enumerate(TOK_TILES):
                ps = pspool.tile([P, C], F32, tag="bigps")
                for kc in range(CT):
                    for (c0, cn) in _chunks(C):
                        nc.tensor.matmul(
                            ps[:tn, c0:c0 + cn],
                            lhsT=xhi_t[kc][:, t0:t0 + tn],
                            rhs=wv_t[kc][:, c0:c0 + cn],
                            start=(kc == 0), stop=(kc == CT - 1))
                vt = vpool.tile([P, C], F16, tag=f"v16_{mt}")
                nc.scalar.copy(vt[:tn, :], ps[:tn, :])
                v16.append(vt)
            # prefetch next batch's x as soon as this batch's is consumed
            if b + 1 < BL:
                next_x = load_x(b + 1)

            # ── head pairs ───────────────────────────────────────────
            rowsum_all = spool.tile([H, N], F32, tag="rowsum_all")
            o16 = [opool.tile([P, N], F16, tag=f"o16_{i}", name=f"o16_{i}")
                   for i in range(CT)]

            def finish_pair(ctx_pair):
                """Row sums + normalize for a pair."""
                hp_, accs_, av_ps_ = ctx_pair
                for par in (0, 1):
                    h = 2 * hp_ + par
                    r0 = par * HD
                    rs_ps = pspool.tile([1, C], F32, tag="bigps",
                                        name=f"rsps{par}")
                    for (c0, cn) in _chunks(N):
                        nc.tensor.matmul(
                            rs_ps[:1, c0:c0 + cn], lhsT=ones128[:],
                            rhs=accs_[h][:, c0:c0 + cn],
                            start=True, stop=True)
                    rs_sb = spool.tile([1, N], F32, tag=f"rs_sb{par}",
                                       name=f"rs_sb{par}")
                    nc.scalar.copy(rs_sb[:], rs_ps[:1, :N])
                    nc.scalar.dma_start(rowsum_all[h:h + 1, :], rs_sb[:])
                    rec = spool.tile([1, N], F32, tag=f"rec{par}",
                                     name=f"rec{par}")
                    nc.vector.reciprocal_approx_fast(rec[:], rs_sb[:])
                    bc = spool.tile([P, NPAD], F32, tag="bcscr",
                                    name=f"bc{par}", bufs=2)
                    nc.gpsimd.partition_broadcast(
                        bc[:r0 + HD, :N], rec[:], channels=r0 + HD)
                    nc.vector.tensor_tensor(
                        o16[hp_][r0:r0 + HD, :], av_ps_[r0:r0 + HD, :N],
                        bc[r0:r0 + HD, :N], OP.mult)

            for hp in range(H // 2):
                p16 = {}
                accs = {}
                # exact scores + exp per head (2 stacked fp16 matmuls)
                for mt, (t0, tn) in enumerate(TOK_TILES):
                    s_ps = {}
                    for par in (0, 1):
                        h = 2 * hp + par
                        ps = pspool.tile([P, C], F32, tag="bigps",
                                         name=f"sps{par}")
                        for (c0, cn) in _chunks(N):
                            nc.tensor.matmul(
                                ps[:tn, c0:c0 + cn],
                                lhsT=khl[h][:, t0:t0 + tn],
                                rhs=qrep[h][:, c0:c0 + cn],
                                start=True, stop=False)
                            nc.tensor.matmul(
                                ps[:tn, c0:c0 + cn],
                                lhsT=khl[h][:, t0:t0 + tn],
                                rhs=qrep[h][:, N + c0:N + c0 + cn],
                                start=False, stop=True)
                        s_ps[par] = ps
                    for par in (0, 1):
                        h = 2 * hp + par
                        if mt == 0:
                            # exp writes the accumulator directly — no copy
                            a0 = ppool.tile([P, N], F32, tag=f"acc{par}",
                                            name=f"acc{par}", bufs=2)
                            nc.scalar.activation(a0[:tn, :],
                                                 s_ps[par][:tn, :N],
                                                 ACTF.Exp, scale=SCALE)
                            accs[h] = a0
                            p32 = a0
                        else:
                            p32 = ppool.tile([P, N], F32, tag="p32",
                                             bufs=3)
                            nc.scalar.activation(p32[:tn, :],
                                                 s_ps[par][:tn, :N],
                                                 ACTF.Exp, scale=SCALE)
                        pt = ppool.tile([P, N], F16,
                                        tag=f"p16_{par}_{mt}",
                                        name=f"p16_{par}_{mt}", bufs=1)
                        # split casts between DVE and ACT to balance engines
                        if mt % 2 == par:
                            nc.vector.tensor_copy(pt[:tn, :], p32[:tn, :])
                        else:
                            nc.scalar.copy(pt[:tn, :], p32[:tn, :])
                        p16[(par, mt)] = pt
                        if mt > 0:
                            nc.vector.tensor_add(accs[h][:tn, :],
                                                 accs[h][:tn, :],
                                                 p32[:tn, :])
                # attn @ V for both heads, column-group packed
                av_ps = psav.tile([P, NPAD], F32, tag="avps")
                for mt, (t0, tn) in enumerate(TOK_TILES):
                    for par in (0, 1):
                        h = 2 * hp + par
                        r0 = par * HD
                        for (c0, cn) in _chunks(N):
                            nc.tensor.matmul(
                                av_ps[r0:r0 + HD, c0:c0 + cn],
                                lhsT=v16[mt][:tn, h * HD:(h + 1) * HD],
                                rhs=p16[(par, mt)][:tn, c0:c0 + cn],
                                start=(mt == 0),
                                stop=(mt == len(TOK_TILES) - 1),
                                tile_position=(0, r0),
                                skip_group_check=True)
                finish_pair((hp, accs, av_ps))

            def emit_scatter(mt):
                t0, tn = TOK_TILES[mt]
                nc.gpsimd.indirect_dma_start(
                    out=out_flat,
                    out_offset=bass.IndirectOffsetOnAxis(
                        ap=icpu_box[0][:tn, mt:mt + 1], axis=0),
                    in_=y1s[mt][:tn, :],
                    in_offset=None,
                    bounds_check=BL * num_kept - 1,
                    oob_is_err=False)

            def emit_yproj(scatter_now):
                # ── output projection + residual (PE keeps running);
                # each tile is scattered as soon as it is ready ──────────
                for mt, (t0, tn) in enumerate(TOK_TILES):
                    y_ps = pspool.tile([P, C], F32, tag="bigps")
                    for kc in range(CT):
                        for (c0, cn) in _chunks(C):
                            nc.tensor.matmul(
                                y_ps[:tn, c0:c0 + cn],
                                lhsT=o16[kc][:, t0:t0 + tn],
                                rhs=wp_t[kc][:, c0:c0 + cn],
                                start=(kc == 0), stop=(kc == CT - 1))
                    xr_t = ypool.tile([P, C], F16, tag="xr_t", bufs=2)
                    nc.sync.dma_start(xr_t[:tn, :], xr_e.ap()[b, t0:t0 + tn, :])
                    y1 = ypool.tile([P, C], F16, tag=f"y1_{mt}",
                                    name=f"y1_{mt}", bufs=1)
                    nc.vector.tensor_add(y1[:tn, :], y_ps[:tn, :], xr_t[:tn, :])
                    y1s.append(y1)
                    if scatter_now:
                        emit_scatter(mt)


            def emit_rank():
                # ── ranking chain (DVE/GpSimd/DMA only — no PE stalls) ───
                pd = spool.tile([H, N], F32, tag="pd")
                nc.scalar.activation(pd[:], sd_sb[:], ACTF.Exp, scale=SCALE)
                rrec = spool.tile([H, N], F32, tag="rrec")
                rscr = spool.tile([H, N], F32, tag="abc")
                nc.vector.reciprocal_approx_accurate(rrec[:], rowsum_all[:],
                                                     rscr[:])
                nc.vector.tensor_mul(pd[:], pd[:], rrec[:])
                a_red = spool.tile([H, N], F32, tag="sd_sb")
                nc.gpsimd.partition_all_reduce(
                    a_red[:], pd[:], channels=H, reduce_op=bass_isa.ReduceOp.add)
                a_row = spool.tile([1, NPAD], F32, tag="a_row")
                nc.vector.tensor_copy(a_row[:, :N], a_red[0:1, :])
                nc.vector.memset(a_row[:, N:], NEG)
                nc.vector.memset(a_row[:, 0:1], 1.0e30)   # CLS always kept

                abc = spool.tile([P, NPAD], F32, tag="abc")
                nc.gpsimd.partition_broadcast(abc[:], a_row[:])
                a_dram = dpool.tile([1, NPAD], F32, tag="a_dram")
                nc.gpsimd.dma_start(a_dram[:], a_row[:])
                acp = spool.tile([P, 5], F32, tag="acp")
                nc.gpsimd.dma_start(
                    acp[:], a_dram[:, :].rearrange("a (t p) -> (a p) t", p=P))
                rcnt = spool.tile([P, 5], F32, tag="rcnt")
                scratch = spool.tile([P, NPAD], F32, tag="bcscr", bufs=2)
                keep16 = spool.tile([P, 5], F16, tag="keep")
                for t in range(5):
                    nc.vector.tensor_scalar(
                        scratch[:], abc[:], acp[:, t:t + 1], None, OP.is_gt,
                        op1=OP.add, accum_out=rcnt[:, t:t + 1])
                    nc.vector.tensor_single_scalar(
                        keep16[:, t:t + 1], rcnt[:, t:t + 1], float(num_kept),
                        OP.is_lt)
                # inclusive prefix over token index j = t*128 + p, all in
                # partition layout: per-tile partition prefix via a
                # triangular-ones matmul, plus a 5-wide scan of tile sums
                pos_ps = psav.tile([P, NPAD], F32, tag="avps")
                nc.tensor.matmul(pos_ps[:, 0:5], lhsT=tri_t[:],
                                 rhs=keep16[:, 0:5], start=True, stop=True)
                nc.tensor.matmul(pos_ps[0:1, 8:13], lhsT=tri_t[:, 127:128],
                                 rhs=keep16[:, 0:5], start=True, stop=True)
                csum = spool.tile([1, 16], F32, tag="csum")
                nc.scalar.copy(csum[:, 0:5], pos_ps[0:1, 8:13])
                nc.vector.tensor_tensor_scan(
                    csum[:, 8:13], csum[:, 0:5], zrow[:, 0:5], 0.0,
                    OP.add, OP.add)
                nc.vector.tensor_tensor(csum[:, 8:13], csum[:, 8:13],
                                        csum[:, 0:5], OP.subtract)
                offs = spool.tile([P, 5], F32, tag="offs")
                nc.gpsimd.partition_broadcast(offs[:], csum[:, 8:13])
                pos = spool.tile([P, 5], F32, tag="pos")
                nc.vector.tensor_tensor(pos[:], pos_ps[:, 0:5], offs[:],
                                        OP.add)
                # scatter index: kept -> b*num_kept + pos - 1, dropped -> BIG
                nc.vector.tensor_single_scalar(
                    pos[:], pos[:], float(b * num_kept - 1), OP.add)
                kbig = spool.tile([P, 5], F32, tag="kbig")
                nc.vector.tensor_scalar(
                    kbig[:], keep16[:], -BIG, BIG, OP.mult, op1=OP.add)
                nc.vector.tensor_tensor(pos[:], pos[:], kbig[:], OP.add)
                icpu = spool.tile([P, 5], U32, tag="icpu")
                nc.vector.tensor_copy(icpu[:], pos[:])
                icpu_box[0] = icpu


            icpu_box = [None]
            y1s = []
            if b == BL - 1:
                # last batch: start the serial rank chain as early as
                # possible; yproj overlaps it and scatters immediately
                emit_rank()
                emit_yproj(scatter_now=True)
            else:
                # middle batches: yproj first so its DVE adds are not
                # queued behind the serial rank chain, then pre-roll the
                # next batch's first two projection blocks so the PE (and
                # the PSUM-releasing ACT/DVE ops) never wait on the rank
                # chain's DMA latency; scatters trail the rank chain
                emit_yproj(scatter_now=False)
                carried = new_proj_state()
                for mo in (0, 1):
                    emit_proj_mo(carried, next_x, mo)
                emit_rank()
                for mt in range(len(TOK_TILES)):
                    emit_scatter(mt)

    n_removed = _dedupe_ldweights(nc)
    nc.compile()
    return nc


def prep_inputs(x, qkv_w, proj_w, proj_b):
    """Host-side sharding + layout prep. Returns per-core in_maps."""
    x = np.ascontiguousarray(x, dtype=np.float32)
    qkv_w = np.asarray(qkv_w, dtype=np.float32)
    proj_w = np.asarray(proj_w, dtype=np.float32)
    proj_b = np.asarray(proj_b, dtype=np.float32)

    wq = np.ascontiguousarray(qkv_w[0:C].T)           # (in_chan, out_chan)
    wk = np.ascontiguousarray(qkv_w[C:2 * C].T)
    whiq = wq.astype(np.float16)
    wloq = (wq - whiq.astype(np.float32)).astype(np.float16)
    whik = wk.astype(np.float16)
    wlok = (wk - whik.astype(np.float32)).astype(np.float16)
    wv16 = np.ascontiguousarray(qkv_w[2 * C:3 * C].T).astype(np.float16)
    wp16 = np.ascontiguousarray(proj_w.T).astype(np.float16)
    hsel = np.zeros((C, H), dtype=np.float32)
    for h in range(H):
        hsel[h * HD:(h + 1) * HD, h] = 1.0
    tri = np.tril(np.ones((P, P), dtype=np.float16)).T.astype(np.float16)

    in_maps = []
    for core in range(NCORES):
        xl = x[core * BL:(core + 1) * BL]             # (BL, N, C)
        xt = np.ascontiguousarray(xl.transpose(0, 2, 1))
        xhi = xt.astype(np.float16)
        xlo = (xt - xhi.astype(np.float32)).astype(np.float16)
        in_maps.append({
            "xhi": xhi,
            "xlo": xlo,
            "xr": (xl + proj_b[None, None, :]).astype(np.float16),
            "whiq": whiq, "wloq": wloq, "whik": whik, "wlok": wlok,
            "wv16": wv16, "wp16": wp16, "hsel": hsel, "tri": tri,
        })
    return in_maps


_BUILD_CACHE = {}


def run(x, qkv_w, proj_w, proj_b, reduction_num, trace=False, **trace_kw):
    num_kept = N - int(reduction_num)
    if num_kept not in _BUILD_CACHE:
        _BUILD_CACHE[num_kept] = build(num_kept)
    nc = _BUILD_CACHE[num_kept]
    in_maps = prep_inputs(x, qkv_w, proj_w, proj_b)
    res = run_bass_kernel_spmd(nc, in_maps, core_ids=list(range(NCORES)),
                               trace=trace, **trace_kw)
    out = np.concatenate([res.results[c]["out"] for c in range(NCORES)],
                         axis=0)
    return out.astype(np.float32), res


def kernel(x, qkv_w, proj_w, proj_b, reduction_num):
    out, _ = run(x, qkv_w, proj_w, proj_b, reduction_num, trace=False)
    return out


# revision 37
# speedup vs baseline: 1.0066x; 1.0066x over previous
"""Trainium2 Bass kernel for nn_Attention_27943057228498 (sparse token-pruning
attention, ViT-style EViT).

Strategy: pure data parallelism over batch — 32 batches over 8 NeuronCores,
4 per core, no collectives.

Numerics: the top-k token selection compares attention diagonal values whose
boundary gaps are as small as ~6e-6 relative, so everything feeding the
ranking (QK projection, scores, softmax row sums, diagonal) must be accurate
to ~1e-6. fp32 matmuls cost 4 cycles/row on the PE (2 half-speed passes), so
instead the exact math runs as fp16 hi/lo split products at 1 cycle/row:

  - Q/K projection: q = whi@xhi + wlo@xhi + whi@xlo  (3 products; the
    dropped wlo@xlo term is ~2^-22 relative).  HW-measured rms error of
    this scheme vs fp64: 1.1e-6 (vs 1.6e-7 for fp32 matmul) — safely
    below the 6e-6 ranking gap.
  - scores: the fp32 q from PSUM is split exactly into qhi + qlo (fp16
    pair); k likewise into a stacked tile khl = [khi; klo].  Per head:
    s = khl^T @ [qhi; qhi] + khl^T @ [qlo; qlo]
      = (khi+klo)^T (qhi+qlo)   — the exact 4-product expansion,
    two K=128 fp16 matmuls per (head, token-tile) instead of the fp32
    form (which costs 4 cycles/row and gets no tile_position
    concurrency on real HW).
  - attention diagonal (ranking signal): qkm = ps_q * k32 with ps_q read
    directly from the projection PSUM (exact fp32), reduced per head with
    a one-hot selector matmul.
  - row sums: fp32 accumulator over exp tiles + ones^T matmul (exact).

The output path (V projection, attn@V, output projection) only needs ~1e-3,
so it runs in plain fp16; the residual add and the final output are fp16
as well (5e-4 rounding on a 2e-2 budget).

Token selection without sorting: per batch, token score a_j gets rank
R_j = #{i: a_i > a_j} via compare + row-reduce; keep = R < num_kept (CLS is
forced kept by pinning a_0 = +1e30); output positions are an inclusive
prefix sum over token index computed entirely in the [128 x 5] partition
layout: a triangular-ones matmul gives per-tile partition prefixes and a
5-wide scan of the tile sums supplies the cross-tile offsets (this avoids
two high-latency DRAM round-trips that a row-layout scan would need).
Rows are emitted with an indirect-DMA scatter whose out-of-bounds indices
(dropped tokens) are silently discarded.
"""

import numpy as np

import concourse.bass as bass
import concourse.bass_isa as bass_isa
import concourse.tile as tile
import concourse.mybir as mybir
from concourse import bacc
from concourse.bass_utils import run_bass_kernel_spmd

# ── problem constants ────────────────────────────────────────────────
B, N, C = 32, 577, 768
H = 12
HD = C // H              # 64
NCORES = 8
BL = B // NCORES         # 4 batches per core
SCALE = HD ** -0.5       # 0.125 (exact power of two)

P = 128
TOK_TILES = [(0, 128), (128, 128), (256, 128), (384, 128), (512, 65)]  # 577
CT = C // P              # 6 channel tiles
NPAD = 640               # 577 padded to 5*128 for the rank machinery
BIG = 1.0e9              # scatter index for dropped rows (exact in fp32)
NEG = -1.0e30            # pad value below any real score

F32 = mybir.dt.float32
F16 = mybir.dt.float16
U32 = mybir.dt.uint32
OP = mybir.AluOpType
ACTF = mybir.ActivationFunctionType


def _dedupe_ldweights(nc):
    """Remove back-to-back duplicate PE Ldweights (same weights AP + array
    tile) so repeated matmuls on one stationary operand pay one load.

    Only deletes an Ldweights when (a) it has no semaphore waits/updates of
    its own, and (b) the PE weight state for its array-tile region is
    provably identical (no intervening Ldweights / self-loading Matmult
    overlapping that region).
    """

    def region(inst):
        tp = inst.tile_position or (0, 0)
        ts = inst.tile_size or (128, 128)
        return (tp[0], tp[0] + ts[0], tp[1], tp[1] + ts[1])

    def overlaps(r1, r2):
        return r1[0] < r2[1] and r2[0] < r1[1] and r1[2] < r2[3] and r2[2] < r1[3]

    removed = 0
    for blk in nc.m.functions[0].blocks:
        state = []   # list of (region, signature)
        keep_list = []
        for inst in blk.instructions:
            if not isinstance(inst, (mybir.InstLdweights, mybir.InstMatmult)):
                keep_list.append(inst)
                continue
            if isinstance(inst, mybir.InstLdweights):
                sig = (str(inst.ins[0]), inst.tile_position, inst.tile_size,
                       inst.is_transpose)
                r = region(inst)
                if (not inst.has_wait() and not inst.has_update()
                        and any(overlaps(r, r2) and s2 == sig and r2 == r
                                for r2, s2 in state)):
                    removed += 1
                    continue     # drop duplicate load
                state = [(r2, s2) for r2, s2 in state if not overlaps(r, r2)]
                state.append((r, sig))
                keep_list.append(inst)
            else:
                # self-loading matmul (fp32) clobbers its region
                if getattr(inst, "ldweights", None) is not False:
                    r = region(inst)
                    state = [(r2, s2) for r2, s2 in state
                             if not overlaps(r, r2)]
                keep_list.append(inst)
        if removed:
            blk.instructions[:] = keep_list
    return removed


def _chunks(total, limit=512):
    out = []
    c0 = 0
    while c0 < total:
        cn = min(limit, total - c0)
        out.append((c0, cn))
        c0 += cn
    return out


def build(num_kept: int):
    nc = bacc.Bacc("TRN2", target_bir_lowering=False, debug=False,
                   num_devices=NCORES)

    xhi_e = nc.dram_tensor("xhi", [BL, C, N], F16, kind="ExternalInput")
    xlo_e = nc.dram_tensor("xlo", [BL, C, N], F16, kind="ExternalInput")
    xr_e = nc.dram_tensor("xr", [BL, N, C], F16, kind="ExternalInput")
    whiq_e = nc.dram_tensor("whiq", [C, C], F16, kind="ExternalInput")
    wloq_e = nc.dram_tensor("wloq", [C, C], F16, kind="ExternalInput")
    whik_e = nc.dram_tensor("whik", [C, C], F16, kind="ExternalInput")
    wlok_e = nc.dram_tensor("wlok", [C, C], F16, kind="ExternalInput")
    wv16_e = nc.dram_tensor("wv16", [C, C], F16, kind="ExternalInput")
    wp16_e = nc.dram_tensor("wp16", [C, C], F16, kind="ExternalInput")
    hsel_e = nc.dram_tensor("hsel", [C, H], F32, kind="ExternalInput")
    tri_e = nc.dram_tensor("tri", [P, P], F16, kind="ExternalInput")
    out_e = nc.dram_tensor("out", [BL, num_kept, C], F16, kind="ExternalOutput")
    out_flat = out_e.ap().rearrange("b n c -> (b n) c")

    from contextlib import ExitStack
    with tile.TileContext(nc) as tc, ExitStack() as ctx:
        wpool = ctx.enter_context(tc.tile_pool(name="weights", bufs=1))
        xpool = ctx.enter_context(tc.tile_pool(name="x", bufs=1))
        qkpool = ctx.enter_context(tc.tile_pool(name="qk", bufs=1))
        qtpool = ctx.enter_context(tc.tile_pool(name="qtmp", bufs=1))
        vpool = ctx.enter_context(tc.tile_pool(name="v", bufs=1))
        opool = ctx.enter_context(tc.tile_pool(name="o", bufs=1))
        ppool = ctx.enter_context(tc.tile_pool(name="p", bufs=2))
        spool = ctx.enter_context(tc.tile_pool(name="small", bufs=1))
        ypool = ctx.enter_context(tc.tile_pool(name="y", bufs=2))
        dpool = ctx.enter_context(tc.tile_pool(name="dram", bufs=2,
                                               space="DRAM"))
        pspool = ctx.enter_context(tc.tile_pool(name="ps", bufs=3,
                                                space="PSUM"))
        psav = ctx.enter_context(tc.tile_pool(name="psav", bufs=1,
                                              space="PSUM"))

        # ── resident weights / constants ─────────────────────────────
        whiq_t, wloq_t, whik_t, wlok_t = [], [], [], []
        wv_t, wp_t, hsel_t = [], [], []
        def load_weights():
            for name, src, dst in (("whiq", whiq_e, whiq_t),
                                   ("wloq", wloq_e, wloq_t),
                                   ("whik", whik_e, whik_t),
                                   ("wlok", wlok_e, wlok_t)):
                for i in range(CT):
                    w = wpool.tile([P, C], F16, tag=f"{name}{i}",
                                   name=f"{name}{i}")
                    nc.scalar.dma_start(w[:], src.ap()[i * P:(i + 1) * P, :])
                    dst.append(w)
            for i in range(CT):
                w5 = wpool.tile([P, H], F32, tag=f"hs{i}", name=f"hs{i}")
                nc.sync.dma_start(w5[:], hsel_e.ap()[i * P:(i + 1) * P, :])
                hsel_t.append(w5)
            for i in range(CT):
                w3 = wpool.tile([P, C], F16, tag=f"wv{i}", name=f"wv{i}")
                nc.scalar.dma_start(w3[:], wv16_e.ap()[i * P:(i + 1) * P, :])
                wv_t.append(w3)
            for i in range(CT):
                w4 = wpool.tile([P, C], F16, tag=f"wp{i}", name=f"wp{i}")
                nc.scalar.dma_start(w4[:], wp16_e.ap()[i * P:(i + 1) * P, :])
                wp_t.append(w4)
        zrow = wpool.tile([1, NPAD], F32, tag="zrow")
        nc.vector.memset(zrow[:], 0.0)
        tri_t = wpool.tile([P, P], F16, tag="tri")
        nc.sync.dma_start(tri_t[:], tri_e.ap()[:, :])
        ones128 = wpool.tile([P, 1], F32, tag="ones128")
        nc.vector.memset(ones128[:], 1.0)

        def load_x(b):
            hi, lo = [], []
            for i in range(CT):
                t1 = xpool.tile([P, N], F16, tag=f"xhi{i}", name=f"xhi{i}")
                nc.sync.dma_start(t1[:], xhi_e.ap()[b, i * P:(i + 1) * P, :])
                hi.append(t1)
            for i in range(CT):
                t2 = xpool.tile([P, N], F16, tag=f"xlo{i}", name=f"xlo{i}")
                nc.sync.dma_start(t2[:], xlo_e.ap()[b, i * P:(i + 1) * P, :])
                lo.append(t2)
            return hi, lo

        # ── Q/K projection emission, one 128-channel (2-head) block ──
        # Per mo-tile: q PSUM then k PSUM; the exact diagonal signal
        # qkm = ps_q * k32 is reduced with the hsel selector matmul
        # while ps_q is alive.  q splits into a combined hi|lo tile
        # (hi = cols 0:N, lo = cols N:2N) that is DMA-replicated per
        # head into qrep = [[qhi|qlo]; [qhi|qlo]]; k splits directly
        # into stacked [khi;klo] (or [klo;khi]) tiles.
        def new_proj_state():
            sd_ps = psav.tile([P, NPAD], F32, tag="avps", name="sd_ps")
            return {"qrep": {}, "khl": {}, "sd_ps": sd_ps}

        def emit_proj_mo(stt, xpair, mo):
            xhi_t, xlo_t = xpair
            qrep, khl, sd_ps = stt["qrep"], stt["khl"], stt["sd_ps"]

            def proj_ps(w_hi, w_lo):
                ps = pspool.tile([P, C], F32, tag="bigps")
                for prod, (wt, xt) in enumerate(
                        ((w_hi, xhi_t), (w_lo, xhi_t), (w_hi, xlo_t))):
                    for kc in range(CT):
                        for (c0, cn) in _chunks(N):
                            nc.tensor.matmul(
                                ps[:, c0:c0 + cn],
                                lhsT=wt[kc][:, mo * P:(mo + 1) * P],
                                rhs=xt[kc][:, c0:c0 + cn],
                                start=(prod == 0 and kc == 0),
                                stop=(prod == 2 and kc == CT - 1))
                return ps

            if True:
                ps_q = proj_ps(whiq_t, wloq_t)
                ps_k = proj_ps(whik_t, wlok_t)

                # exact diagonal contribution from this channel block
                # (DVE cannot read two PSUM operands, so stage k in SBUF)
                k32 = qtpool.tile([P, N], F32, tag="k32", name=f"k32_{mo}")
                nc.scalar.copy(k32[:], ps_k[:, :N])
                qkm = qkpool.tile([P, NPAD], F32, tag="qkm", bufs=1)
                nc.vector.tensor_mul(qkm[:, :N], ps_q[:, :N], k32[:])
                for (c0, cn) in _chunks(N):
                    nc.tensor.matmul(
                        sd_ps[:H, c0:c0 + cn],
                        lhsT=hsel_t[mo][:],
                        rhs=qkm[:, c0:c0 + cn],
                        start=(mo == 0), stop=(mo == CT - 1))

                # q: hi/lo split into one combined tile (hi|lo columns),
                # then per-head replication with 2 DMAs per head
                qhl = qtpool.tile([P, 2 * N], F16, tag="qhl", bufs=2,
                                  name=f"qhl{mo}")
                nc.scalar.copy(qhl[:, 0:N], ps_q[:, :N])
                nc.vector.tensor_tensor(qhl[:, N:2 * N], ps_q[:, :N],
                                        qhl[:, 0:N], OP.subtract)
                for par in (0, 1):
                    h = 2 * mo + par
                    r = par * HD
                    t_r = qkpool.tile([P, 2 * N], F16, tag=f"qrep{h}",
                                      name=f"qrep{h}")
                    nc.sync.dma_start(t_r[0:HD, :], qhl[r:r + HD, :])
                    nc.sync.dma_start(t_r[HD:P, :], qhl[r:r + HD, :])
                    qrep[h] = t_r

                # k: stacked hi/lo per head.  Even head: [khi; klo] with
                # the hi half a lanewise copy; odd head: [klo; khi].
                h0, h1 = 2 * mo, 2 * mo + 1
                st0 = qkpool.tile([P, N], F16, tag=f"khl{h0}",
                                  name=f"khl{h0}")
                st1 = qkpool.tile([P, N], F16, tag=f"khl{h1}",
                                  name=f"khl{h1}")
                nc.scalar.copy(st0[0:HD, :], ps_k[0:HD, :N])
                nc.scalar.copy(st1[HD:P, :], ps_k[HD:P, :N])
                k_lo = qtpool.tile([P, N], F16, tag="k_lo", bufs=2,
                                   name=f"klo{mo}")
                nc.vector.tensor_tensor(k_lo[0:HD, :], ps_k[0:HD, :N],
                                        st0[0:HD, :], OP.subtract)
                nc.vector.tensor_tensor(k_lo[HD:P, :], ps_k[HD:P, :N],
                                        st1[HD:P, :], OP.subtract)
                nc.scalar.dma_start(st0[HD:P, :], k_lo[0:HD, :])
                nc.scalar.dma_start(st1[0:HD, :], k_lo[HD:P, :])
                khl[h0] = st0
                khl[h1] = st1

        next_x = load_x(0)
        load_weights()
        carried = None
        for b in range(BL):
            xhi_t, xlo_t = next_x
            if carried is None:
                stt = new_proj_state()
                mos = range(CT)
            else:
                stt = carried
                carried = None
                mos = range(3, CT)
            for mo in mos:
                emit_proj_mo(stt, (xhi_t, xlo_t), mo)
            qrep, khl, sd_ps = stt["qrep"], stt["khl"], stt["sd_ps"]

            sd_sb = spool.tile([H, N], F32, tag="sd_sb")
            nc.scalar.copy(sd_sb[:], sd_ps[:H, :N])

            # ── V projection (fp16) → v16[mt] = (m, C) ───────────────
            v16 = []
            for mt, (t0, tn) in enumerate(TOK_TILES):
                ps = pspool.tile([P, C], F32, tag="bigps")
                for kc in range(CT):
                    for (c0, cn) in _chunks(C):
                        nc.tensor.matmul(
                            ps[:tn, c0:c0 + cn],
                            lhsT=xhi_t[kc][:, t0:t0 + tn],
                            rhs=wv_t[kc][:, c0:c0 + cn],
                            start=(kc == 0), stop=(kc == CT - 1))
                vt = vpool.tile([P, C], F16, tag=f"v16_{mt}")
                nc.scalar.copy(vt[:tn, :], ps[:tn, :])
                v16.append(vt)
            # prefetch next batch's x as soon as this batch's is consumed
            if b + 1 < BL:
                next_x = load_x(b + 1)

            # ── head pairs ───────────────────────────────────────────
            rowsum_all = spool.tile([H, N], F32, tag="rowsum_all")
            o16 = [opool.tile([P, N], F16, tag=f"o16_{i}", name=f"o16_{i}")
                   for i in range(CT)]

            def finish_pair(ctx_pair):
                """Row sums + normalize for a pair."""
                hp_, accs_, av_ps_ = ctx_pair
                for par in (0, 1):
                    h = 2 * hp_ + par
                    r0 = par * HD
                    rs_ps = pspool.tile([1, C], F32, tag="bigps",
                                        name=f"rsps{par}")
                    for (c0, cn) in _chunks(N):
                        nc.tensor.matmul(
                            rs_ps[:1, c0:c0 + cn], lhsT=ones128[:],
                            rhs=accs_[h][:, c0:c0 + cn],
                            start=True, stop=True)
                    rs_sb = spool.tile([1, N], F32, tag=f"rs_sb{par}",
                                       name=f"rs_sb{par}")
                    nc.scalar.copy(rs_sb[:], rs_ps[:1, :N])
                    nc.scalar.dma_start(rowsum_all[h:h + 1, :], rs_sb[:])
                    rec = spool.tile([1, N], F32, tag=f"rec{par}",
                                     name=f"rec{par}")
                    nc.vector.reciprocal_approx_fast(rec[:], rs_sb[:])
                    bc = spool.tile([P, NPAD], F32, tag="bcscr",
                                    name=f"bc{par}", bufs=2)
                    nc.gpsimd.partition_broadcast(
                        bc[:r0 + HD, :N], rec[:], channels=r0 + HD)
                    nc.vector.tensor_tensor(
                        o16[hp_][r0:r0 + HD, :], av_ps_[r0:r0 + HD, :N],
                        bc[r0:r0 + HD, :N], OP.mult)

            for hp in range(H // 2):
                p16 = {}
                accs = {}
                # exact scores + exp per head (2 stacked fp16 matmuls)
                for mt, (t0, tn) in enumerate(TOK_TILES):
                    s_ps = {}
                    for par in (0, 1):
                        h = 2 * hp + par
                        ps = pspool.tile([P, C], F32, tag="bigps",
                                         name=f"sps{par}")
                        for (c0, cn) in _chunks(N):
                            nc.tensor.matmul(
                                ps[:tn, c0:c0 + cn],
                                lhsT=khl[h][:, t0:t0 + tn],
                                rhs=qrep[h][:, c0:c0 + cn],
                                start=True, stop=False)
                            nc.tensor.matmul(
                                ps[:tn, c0:c0 + cn],
                                lhsT=khl[h][:, t0:t0 + tn],
                                rhs=qrep[h][:, N + c0:N + c0 + cn],
                                start=False, stop=True)
                        s_ps[par] = ps
                    for par in (0, 1):
                        h = 2 * hp + par
                        if mt == 0:
                            # exp writes the accumulator directly — no copy
                            a0 = ppool.tile([P, N], F32, tag=f"acc{par}",
                                            name=f"acc{par}", bufs=2)
                            nc.scalar.activation(a0[:tn, :],
                                                 s_ps[par][:tn, :N],
                                                 ACTF.Exp, scale=SCALE)
                            accs[h] = a0
                            p32 = a0
                        else:
                            p32 = ppool.tile([P, N], F32, tag="p32",
                                             bufs=3)
                            nc.scalar.activation(p32[:tn, :],
                                                 s_ps[par][:tn, :N],
                                                 ACTF.Exp, scale=SCALE)
                        pt = ppool.tile([P, N], F16,
                                        tag=f"p16_{par}_{mt}",
                                        name=f"p16_{par}_{mt}", bufs=1)
                        # split casts between DVE and ACT to balance engines
                        if mt % 2 == par:
                            nc.vector.tensor_copy(pt[:tn, :], p32[:tn, :])
                        else:
                            nc.scalar.copy(pt[:tn, :], p32[:tn, :])
                        p16[(par, mt)] = pt
                        if mt > 0:
                            nc.vector.tensor_add(accs[h][:tn, :],
                                                 accs[h][:tn, :],
                                                 p32[:tn, :])
                # attn @ V for both heads, column-group packed
                av_ps = psav.tile([P, NPAD], F32, tag="avps")
                for mt, (t0, tn) in enumerate(TOK_TILES):
                    for par in (0, 1):
                        h = 2 * hp + par
                        r0 = par * HD
                        for (c0, cn) in _chunks(N):
                            nc.tensor.matmul(
                                av_ps[r0:r0 + HD, c0:c0 + cn],
                                lhsT=v16[mt][:tn, h * HD:(h + 1) * HD],
                                rhs=p16[(par, mt)][:tn, c0:c0 + cn],
                                start=(mt == 0),
                                stop=(mt == len(TOK_TILES) - 1),
                                tile_position=(0, r0),
                                skip_group_check=True)
                finish_pair((hp, accs, av_ps))

            def emit_scatter(mt):
                t0, tn = TOK_TILES[mt]
                nc.gpsimd.indirect_dma_start(
                    out=out_flat,
                    out_offset=bass.IndirectOffsetOnAxis(
                        ap=icpu_box[0][:tn, mt:mt + 1], axis=0),
                    in_=y1s[mt][:tn, :],
                    in_offset=None,
                    bounds_check=BL * num_kept - 1,
                    oob_is_err=False)

            def emit_yproj(scatter_now):
                # ── output projection + residual (PE keeps running);
                # each tile is scattered as soon as it is ready ──────────
                for mt, (t0, tn) in enumerate(TOK_TILES):
                    y_ps = pspool.tile([P, C], F32, tag="bigps")
                    for kc in range(CT):
                        for (c0, cn) in _chunks(C):
                            nc.tensor.matmul(
                                y_ps[:tn, c0:c0 + cn],
                                lhsT=o16[kc][:, t0:t0 + tn],
                                rhs=wp_t[kc][:, c0:c0 + cn],
                                start=(kc == 0), stop=(kc == CT - 1))
                    xr_t = ypool.tile([P, C], F16, tag="xr_t", bufs=2)
                    nc.sync.dma_start(xr_t[:tn, :], xr_e.ap()[b, t0:t0 + tn, :])
                    y1 = ypool.tile([P, C], F16, tag=f"y1_{mt}",
                                    name=f"y1_{mt}", bufs=1)
                    nc.vector.tensor_add(y1[:tn, :], y_ps[:tn, :], xr_t[:tn, :])
                    y1s.append(y1)
                    if scatter_now:
                        emit_scatter(mt)


            def emit_rank():
                # ── ranking chain (DVE/GpSimd/DMA only — no PE stalls) ───
                pd = spool.tile([H, N], F32, tag="pd")
                nc.scalar.activation(pd[:], sd_sb[:], ACTF.Exp, scale=SCALE)
                rrec = spool.tile([H, N], F32, tag="rrec")
                rscr = spool.tile([H, N], F32, tag="abc")
                nc.vector.reciprocal_approx_accurate(rrec[:], rowsum_all[:],
                                                     rscr[:])
                nc.vector.tensor_mul(pd[:], pd[:], rrec[:])
                a_red = spool.tile([H, N], F32, tag="sd_sb")
                nc.gpsimd.partition_all_reduce(
                    a_red[:], pd[:], channels=H, reduce_op=bass_isa.ReduceOp.add)
                a_row = spool.tile([1, NPAD], F32, tag="a_row")
                nc.vector.tensor_copy(a_row[:, :N], a_red[0:1, :])
                nc.vector.memset(a_row[:, N:], NEG)
                nc.vector.memset(a_row[:, 0:1], 1.0e30)   # CLS always kept

                abc = spool.tile([P, NPAD], F32, tag="abc")
                nc.gpsimd.partition_broadcast(abc[:], a_row[:])
                a_dram = dpool.tile([1, NPAD], F32, tag="a_dram")
                nc.gpsimd.dma_start(a_dram[:], a_row[:])
                acp = spool.tile([P, 5], F32, tag="acp")
                nc.gpsimd.dma_start(
                    acp[:], a_dram[:, :].rearrange("a (t p) -> (a p) t", p=P))
                rcnt = spool.tile([P, 5], F32, tag="rcnt")
                scratch = spool.tile([P, NPAD], F32, tag="bcscr", bufs=2)
                keep16 = spool.tile([P, 5], F16, tag="keep")
                for t in range(5):
                    nc.vector.tensor_scalar(
                        scratch[:], abc[:], acp[:, t:t + 1], None, OP.is_gt,
                        op1=OP.add, accum_out=rcnt[:, t:t + 1])
                    nc.vector.tensor_single_scalar(
                        keep16[:, t:t + 1], rcnt[:, t:t + 1], float(num_kept),
                        OP.is_lt)
                # inclusive prefix over token index j = t*128 + p, all in
                # partition layout: per-tile partition prefix via a
                # triangular-ones matmul, plus a 5-wide scan of tile sums
                pos_ps = psav.tile([P, NPAD], F32, tag="avps")
                nc.tensor.matmul(pos_ps[:, 0:5], lhsT=tri_t[:],
                                 rhs=keep16[:, 0:5], start=True, stop=True)
                nc.tensor.matmul(pos_ps[0:1, 8:13], lhsT=tri_t[:, 127:128],
                                 rhs=keep16[:, 0:5], start=True, stop=True)
                csum = spool.tile([1, 16], F32, tag="csum")
                nc.scalar.copy(csum[:, 0:5], pos_ps[0:1, 8:13])
                nc.vector.tensor_tensor_scan(
                    csum[:, 8:13], csum[:, 0:5], zrow[:, 0:5], 0.0,
                    OP.add, OP.add)
                nc.vector.tensor_tensor(csum[:, 8:13], csum[:, 8:13],
                                        csum[:, 0:5], OP.subtract)
                offs = spool.tile([P, 5], F32, tag="offs")
                nc.gpsimd.partition_broadcast(offs[:], csum[:, 8:13])
                pos = spool.tile([P, 5], F32, tag="pos")
                nc.vector.tensor_tensor(pos[:], pos_ps[:, 0:5], offs[:],
                                        OP.add)
                # scatter index: kept -> b*num_kept + pos - 1, dropped -> BIG
                nc.vector.tensor_single_scalar(
                    pos[:], pos[:], float(b * num_kept - 1), OP.add)
                kbig = spool.tile([P, 5], F32, tag="kbig")
                nc.vector.tensor_scalar(
                    kbig[:], keep16[:], -BIG, BIG, OP.mult, op1=OP.add)
                nc.vector.tensor_tensor(pos[:], pos[:], kbig[:], OP.add)
                icpu = spool.tile([P, 5], U32, tag="icpu")
                nc.vector.tensor_copy(icpu[:], pos[:])
                icpu_box[0] = icpu


            icpu_box = [None]
            y1s = []
            if b == BL - 1:
                # last batch: start the serial rank chain as early as
                # possible; yproj overlaps it and scatters immediately
                emit_rank()
                emit_yproj(scatter_now=True)
            else:
                # middle batches: yproj first so its DVE adds are not
                # queued behind the serial rank chain, then pre-roll the
                # next batch's first two projection blocks so the PE (and
                # the PSUM-releasing ACT/DVE ops) never wait on the rank
                # chain's DMA latency; scatters trail the rank chain
                emit_yproj(scatter_now=False)
                carried = new_proj_state()
                for mo in (0, 1, 2):
                    emit_proj_mo(carried, next_x, mo)
                emit_rank()
                for mt in range(len(TOK_TILES)):
                    emit_scatter(mt)

    n_removed = _dedupe_ldweights(nc)
    nc.compile()
    return nc


def prep_inputs(x, qkv_w, proj_w, proj_b):
    """Host-side sharding + layout prep. Returns per-core in_maps."""
    x = np.ascontiguousarray(x, dtype=np.float32)
    qkv_w = np.asarray(qkv_w, dtype=np.float32)
    proj_w = np.asarray(proj_w, dtype=np.float32)
    proj_b = np.asarray(proj_b, dtype=np.float32)

    wq = np.ascontiguousarray(qkv_w[0:C].T)           # (in_chan, out_chan)
    wk = np.ascontiguousarray(qkv_w[C:2 * C].T)
    whiq = wq.astype(np.float16)
    wloq = (wq - whiq.astype(np.float32)).astype(np.float16)
    whik = wk.astype(np.float16)
    wlok = (wk - whik.astype(np.float32)).astype(np.float16)
    wv16 = np.ascontiguousarray(qkv_w[2 * C:3 * C].T).astype(np.float16)
    wp16 = np.ascontiguousarray(proj_w.T).astype(np.float16)
    hsel = np.zeros((C, H), dtype=np.float32)
    for h in range(H):
        hsel[h * HD:(h + 1) * HD, h] = 1.0
    tri = np.tril(np.ones((P, P), dtype=np.float16)).T.astype(np.float16)

    in_maps = []
    for core in range(NCORES):
        xl = x[core * BL:(core + 1) * BL]             # (BL, N, C)
        xt = np.ascontiguousarray(xl.transpose(0, 2, 1))
        xhi = xt.astype(np.float16)
        xlo = (xt - xhi.astype(np.float32)).astype(np.float16)
        in_maps.append({
            "xhi": xhi,
            "xlo": xlo,
            "xr": (xl + proj_b[None, None, :]).astype(np.float16),
            "whiq": whiq, "wloq": wloq, "whik": whik, "wlok": wlok,
            "wv16": wv16, "wp16": wp16, "hsel": hsel, "tri": tri,
        })
    return in_maps


_BUILD_CACHE = {}


def run(x, qkv_w, proj_w, proj_b, reduction_num, trace=False, **trace_kw):
    num_kept = N - int(reduction_num)
    if num_kept not in _BUILD_CACHE:
        _BUILD_CACHE[num_kept] = build(num_kept)
    nc = _BUILD_CACHE[num_kept]
    in_maps = prep_inputs(x, qkv_w, proj_w, proj_b)
    res = run_bass_kernel_spmd(nc, in_maps, core_ids=list(range(NCORES)),
                               trace=trace, **trace_kw)
    out = np.concatenate([res.results[c]["out"] for c in range(NCORES)],
                         axis=0)
    return out.astype(np.float32), res


def kernel(x, qkv_w, proj_w, proj_b, reduction_num):
    out, _ = run(x, qkv_w, proj_w, proj_b, reduction_num, trace=False)
    return out
